# revision 3
# baseline (speedup 1.0000x reference)
"""Multi-head attention (B=2, S=2048, D=1024, H=16, dh=64) on 8 Trainium2 cores.

Sharding: head-tensor-parallel x batch. Core c owns batch b=c//4 and heads
4*(c%4)..4*(c%4)+3 (256 of the 1024 ctx dims). Each core computes its heads'
Q/K/V projections, attention, and a partial output projection against its
256 rows of Wo (+ bo/4 so the 4 partials per batch sum to one bo). The host
unshard step sums the 4 partial outputs per batch — the tensor-parallel
all-reduce of the sharding hint, done at gather time.

Per-core kernel layout (all matmuls bf16 operands, fp32 PSUM accumulation):
  qT/kT [256e, 2048t] = W.T @ x.T computed directly in transposed form so
  scores^T [kt, qt] = (kT slice).T @ (qT slice) needs no on-device transpose;
  exp on ScalarE with the 1/sqrt(dh) scale folded in; A@V uses a stationary
  operand [V | 1] (ones column baked into wv_ext via a zero column + bias
  trick) so the softmax denominator falls out of the same matmul; the
  reciprocal is broadcast across partitions with a K=1 matmul; out^T =
  Wo_slice.T @ ctx_norm^T with bo/4 added as a per-partition bias at PSUM
  eviction.
"""

import numpy as np
import ml_dtypes

import bass_rust
import concourse.bass as bass
import concourse.mybir as mybir
import concourse.tile as tile
from concourse.bass_utils import run_bass_kernel_spmd

B = 2
S = 2048
D = 1024
H = 16
DH = 64
OUT = 1024
NCORES = 8
HPC = H // 4  # heads per core = 4
E = HPC * DH  # 256 ctx dims per core
EV = HPC * (DH + 1)  # 260: v with interleaved ones columns

BF16 = mybir.dt.bfloat16
FP32 = mybir.dt.float32
FP16 = mybir.dt.float16

SCALE = 1.0 / float(np.sqrt(DH))


def _split_waits(nc, maxw=1):
    """This container's walrus rejects instructions carrying more than one
    semaphore wait ("Too many sync wait commands"); hoist extras onto
    standalone same-engine nops, preserving per-engine program order."""
    for bb in nc.main_func.blocks:
        new_il = []
        for inst in bb.instructions:
            si = inst.sync_info
            if si is not None and si.on_wait and len(si.on_wait) > maxw:
                waits = list(si.on_wait)
                for j, w in enumerate(waits[:-maxw]):
                    nop = mybir.InstNoOp(
                        name=f"{inst.name}-ws{j}", ins=[], outs=[], engine=inst.engine
                    )
                    nop.sync_info = bass_rust.SyncInfo(on_wait=[w], on_update=[])
                    new_il.append(nop)
                inst.sync_info = bass_rust.SyncInfo(
                    on_wait=waits[-maxw:], on_update=list(si.on_update)
                )
            new_il.append(inst)
        bb.instructions = new_il


def build_program():
    nc = bass.Bass()

    xT = nc.declare_dram_parameter("xT", [D, S], BF16, isOutput=False)
    wq = nc.declare_dram_parameter("wq", [D, E], BF16, isOutput=False)
    wk = nc.declare_dram_parameter("wk", [D, E], BF16, isOutput=False)
    wv = nc.declare_dram_parameter("wv", [D, EV], BF16, isOutput=False)
    wo = nc.declare_dram_parameter("wo", [E, OUT], BF16, isOutput=False)
    bqp = nc.declare_dram_parameter("bq", [128, E // 128], FP32, isOutput=False)
    bkp = nc.declare_dram_parameter("bk", [128, E // 128], FP32, isOutput=False)
    bvp = nc.declare_dram_parameter("bv", [1, EV], BF16, isOutput=False)
    bop = nc.declare_dram_parameter("bo4", [128, OUT // 128], FP32, isOutput=False)
    outT = nc.declare_dram_parameter("outT", [OUT, S], FP32, isOutput=True)

    KT = D // 128  # 8 k-tiles for projections
    MT = S // 128  # 16 token tiles
    NQ = S // 512  # 4 query slices of 512

    with tile.TileContext(nc) as tc:
        with (
            tc.tile_pool(name="w", bufs=1) as wpool,
            tc.tile_pool(name="work", bufs=3) as work,
            tc.tile_pool(name="cnp", bufs=1) as cnpool,
            tc.tile_pool(name="ps", bufs=2, space="PSUM") as psp,
            tc.tile_pool(name="ctxps", bufs=4, space="PSUM") as ctxp,
        ):
            # ---- persistent SBUF residents ----
            xts = [wpool.tile([128, S], BF16, tag=f"xt{k}", name=f"xt{k}") for k in range(KT)]
            wqs = [wpool.tile([128, E], BF16, tag=f"wq{k}", name=f"wq{k}") for k in range(KT)]
            wks = [wpool.tile([128, E], BF16, tag=f"wk{k}", name=f"wk{k}") for k in range(KT)]
            wvs = [wpool.tile([128, EV], BF16, tag=f"wv{k}", name=f"wv{k}") for k in range(KT)]
            wos = [wpool.tile([128, OUT], BF16, tag=f"wo{k}", name=f"wo{k}") for k in range(2)]
            bq_s = wpool.tile([128, E // 128], FP32, tag="bq")
            bk_s = wpool.tile([128, E // 128], FP32, tag="bk")
            bv_s = wpool.tile([1, EV], BF16, tag="bv")
            bo_s = wpool.tile([128, OUT // 128], FP32, tag="bo")
            ones_b = wpool.tile([1, 128], BF16, tag="ones_b")
            ones_h = wpool.tile([1, 64], FP16, tag="ones_h")
            qts = [wpool.tile([128, S], BF16, tag=f"qt{m}", name=f"qt{m}") for m in range(2)]
            kts = [wpool.tile([128, S], BF16, tag=f"kt{m}", name=f"kt{m}") for m in range(2)]
            vts = [wpool.tile([128, EV], BF16, tag=f"vt{m}", name=f"vt{m}") for m in range(MT)]
            cns = [cnpool.tile([128, S], BF16, tag=f"cn{m}", name=f"cn{m}") for m in range(2)]

            for k in range(KT):
                nc.sync.dma_start(out=xts[k][:], in_=xT[k * 128 : (k + 1) * 128, :])
            for k in range(KT):
                nc.sync.dma_start(out=wqs[k][:], in_=wq[k * 128 : (k + 1) * 128, :])
                nc.sync.dma_start(out=wks[k][:], in_=wk[k * 128 : (k + 1) * 128, :])
                nc.sync.dma_start(out=wvs[k][:], in_=wv[k * 128 : (k + 1) * 128, :])
            for k in range(2):
                nc.sync.dma_start(out=wos[k][:], in_=wo[k * 128 : (k + 1) * 128, :])
            nc.sync.dma_start(out=bq_s[:], in_=bqp[:])
            nc.sync.dma_start(out=bk_s[:], in_=bkp[:])
            nc.sync.dma_start(out=bv_s[:], in_=bvp[:])
            nc.sync.dma_start(out=bo_s[:], in_=bop[:])
            nc.vector.memset(ones_b[:], 1.0)
            nc.vector.memset(ones_h[:], 1.0)

            def qk_proj(hp):
                """qT/kT rows 128*hp..128*hp+127 (heads 2hp, 2hp+1), all tokens."""
                for w_s, dst, bias in ((wqs, qts, bq_s), (wks, kts, bk_s)):
                    for n in range(NQ):
                        ps = psp.tile([128, 512], FP32, tag="S", name="ps")
                        for k in range(KT):
                            nc.tensor.matmul(
                                ps[:],
                                lhsT=w_s[k][:, hp * 128 : (hp + 1) * 128],
                                rhs=xts[k][:, n * 512 : (n + 1) * 512],
                                start=(k == 0),
                                stop=(k == KT - 1),
                            )
                        nc.scalar.activation(
                            dst[hp][:, n * 512 : (n + 1) * 512],
                            ps[:],
                            mybir.ActivationFunctionType.Identity,
                            bias=bq_s[:, hp : hp + 1] if bias is bq_s else bk_s[:, hp : hp + 1],
                        )

            def v_proj():
                for m in range(MT):
                    ps = psp.tile([128, EV], FP32, tag="S", name="psv")
                    for k in range(KT):
                        nc.tensor.matmul(
                            ps[:],
                            lhsT=xts[k][:, m * 128 : (m + 1) * 128],
                            rhs=wvs[k][:],
                            start=(k == 0),
                            stop=False,
                        )
                    # ones columns + v bias: rank-1 [1,128].T @ [1,EV]
                    nc.tensor.matmul(
                        ps[:], lhsT=ones_b[:], rhs=bv_s[:], start=False, stop=True
                    )
                    nc.vector.tensor_copy(vts[m][:], ps[:])

            def normalize(ctx, hp, a, nq):
                """ctx [65,512] psum -> cn[hp] rows 64a..64a+63, cols nq*512.."""
                r = work.tile([1, 512], FP16, tag="recip")
                with nc.allow_low_precision(reason="fp16 recip feeds fp16 K=1 bcast matmul"):
                    nc.vector.reciprocal(r[:], ctx[64:65, :])
                bc = ctxp.tile([65, 512], FP32, tag="ctx")
                nc.tensor.matmul(
                    bc[0:64, :], lhsT=ones_h[:], rhs=r[:], start=True, stop=True
                )
                bcs = work.tile([64, 512], FP32, tag="bcs")
                nc.vector.tensor_copy(bcs[:], bc[0:64, :])
                nc.vector.tensor_mul(
                    cns[hp][64 * a : 64 * a + 64, nq * 512 : (nq + 1) * 512],
                    ctx[0:64, :],
                    bcs[:],
                )

            def attention(hp):
                for nq in range(NQ):
                    ctx_a = ctxp.tile([65, 512], FP32, tag="ctx", name="ctx_a")
                    ctx_b = ctxp.tile([65, 512], FP32, tag="ctx", name="ctx_b")
                    for m in range(MT):
                        sps = psp.tile([128, 1024], FP32, tag="S", name="sps")
                        # scores^T for the head pair (row-packed K=64 matmuls)
                        nc.tensor.matmul(
                            sps[:, 0:512],
                            lhsT=kts[hp][0:64, m * 128 : (m + 1) * 128],
                            rhs=qts[hp][0:64, nq * 512 : (nq + 1) * 512],
                            start=True,
                            stop=True,
                        )
                        nc.tensor.matmul(
                            sps[:, 512:1024],
                            lhsT=kts[hp][64:128, m * 128 : (m + 1) * 128],
                            rhs=qts[hp][64:128, nq * 512 : (nq + 1) * 512],
                            start=True,
                            stop=True,
                        )
                        ee = work.tile([128, 1024], BF16, tag="E")
                        nc.scalar.activation(
                            ee[:], sps[:], mybir.ActivationFunctionType.Exp, scale=SCALE
                        )
                        ha = 2 * hp
                        nc.tensor.matmul(
                            ctx_a[:],
                            lhsT=vts[m][:, ha * 65 : ha * 65 + 65],
                            rhs=ee[:, 0:512],
                            start=(m == 0),
                            stop=(m == MT - 1),
                        )
                        nc.tensor.matmul(
                            ctx_b[:],
                            lhsT=vts[m][:, (ha + 1) * 65 : (ha + 1) * 65 + 65],
                            rhs=ee[:, 512:1024],
                            start=(m == 0),
                            stop=(m == MT - 1),
                        )
                    normalize(ctx_a, hp, 0, nq)
                    normalize(ctx_b, hp, 1, nq)

            def out_proj():
                for mo in range(OUT // 128):
                    for n in range(NQ):
                        ps = psp.tile([128, 512], FP32, tag="S", name="ps")
                        for k in range(2):
                            nc.tensor.matmul(
                                ps[:],
                                lhsT=wos[k][:, mo * 128 : (mo + 1) * 128],
                                rhs=cns[k][:, n * 512 : (n + 1) * 512],
                                start=(k == 0),
                                stop=(k == 1),
                            )
                        ot = work.tile([128, 512], FP32, tag="ot")
                        nc.scalar.activation(
                            ot[:],
                            ps[:],
                            mybir.ActivationFunctionType.Identity,
                            bias=bo_s[:, mo : mo + 1],
                        )
                        nc.sync.dma_start(
                            out=outT[mo * 128 : (mo + 1) * 128, n * 512 : (n + 1) * 512],
                            in_=ot[:],
                        )

            qk_proj(0)
            v_proj()
            attention(0)
            qk_proj(1)
            attention(1)
            out_proj()

    _split_waits(nc)
    return nc


_PROGRAM = None


def _get_program():
    global _PROGRAM
    if _PROGRAM is None:
        _PROGRAM = build_program()
    return _PROGRAM


def _shard_inputs(x, Wq, bq, Wk, bk, Wv, bv, Wo, bo):
    bf16 = ml_dtypes.bfloat16
    in_maps = []
    for c in range(NCORES):
        b = c // 4
        g = c % 4
        hs = slice(g * HPC, (g + 1) * HPC)

        xT = np.ascontiguousarray(x[b].T).astype(bf16)  # [D, S]
        wq_c = np.ascontiguousarray(
            Wq[hs].transpose(1, 0, 2).reshape(D, E)
        ).astype(bf16)
        wk_c = np.ascontiguousarray(
            Wk[hs].transpose(1, 0, 2).reshape(D, E)
        ).astype(bf16)
        wv_c = np.zeros((D, EV), dtype=np.float32)
        bv_c = np.zeros((1, EV), dtype=np.float32)
        for h in range(HPC):
            wv_c[:, h * 65 : h * 65 + 64] = Wv[hs][h]
            bv_c[0, h * 65 : h * 65 + 64] = bv[hs][h]
            bv_c[0, h * 65 + 64] = 1.0
        wo_c = np.ascontiguousarray(Wo[g * E : (g + 1) * E, :]).astype(bf16)
        bq_c = np.ascontiguousarray(
            bq[hs].reshape(E // 128, 128).T
        ).astype(np.float32)
        bk_c = np.ascontiguousarray(
            bk[hs].reshape(E // 128, 128).T
        ).astype(np.float32)
        bo_c = np.ascontiguousarray(
            (bo.astype(np.float32) * 0.25).reshape(OUT // 128, 128).T
        ).astype(np.float32)

        in_maps.append(
            {
                "xT": xT,
                "wq": wq_c,
                "wk": wk_c,
                "wv": wv_c.astype(bf16),
                "wo": wo_c,
                "bq": bq_c,
                "bk": bk_c,
                "bv": bv_c.astype(bf16),
                "bo4": bo_c,
            }
        )
    return in_maps


def kernel(x, Wq, bq, Wk, bk, Wv, bv, Wo, bo, _trace=False, _result_box=None):
    x = np.asarray(x, dtype=np.float32)
    in_maps = _shard_inputs(
        np.asarray(x, np.float32),
        np.asarray(Wq, np.float32),
        np.asarray(bq, np.float32),
        np.asarray(Wk, np.float32),
        np.asarray(bk, np.float32),
        np.asarray(Wv, np.float32),
        np.asarray(bv, np.float32),
        np.asarray(Wo, np.float32),
        np.asarray(bo, np.float32),
    )
    nc = _get_program()
    res = run_bass_kernel_spmd(nc, in_maps, list(range(NCORES)), trace=_trace)
    if _result_box is not None:
        _result_box.append(res)

    out = np.empty((B, S, OUT), dtype=np.float32)
    for b in range(B):
        acc = res.results[4 * b]["outT"].astype(np.float32).copy()
        for g in range(1, 4):
            acc += res.results[4 * b + g]["outT"]
        out[b] = acc.T
    return out


# revision 9
# speedup vs baseline: 1.0538x; 1.0538x over previous
"""Multi-head attention (B=2, S=2048, D=1024, H=16, dh=64) on 8 Trainium2 cores.

Sharding: head-tensor-parallel x batch. Core c owns batch b=c//4 and heads
4*(c%4)..4*(c%4)+3 (256 of the 1024 ctx dims). Each core computes its heads'
Q/K/V projections, attention, and a partial output projection against its
256 rows of Wo (+ bo/4 so the 4 partials per batch sum to one bo). The host
unshard step sums the 4 partial outputs per batch — the tensor-parallel
all-reduce of the sharding hint, done at gather time.

Per-core kernel layout (all matmuls bf16 operands, fp32 PSUM accumulation):
  qT/kT [256e, 2048t] = W.T @ x.T computed directly in transposed form so
  scores^T [kt, qt] = (kT slice).T @ (qT slice) needs no on-device transpose;
  exp on ScalarE with the 1/sqrt(dh) scale folded in; A@V uses a stationary
  operand [V | 1] (ones column baked into wv_ext via a zero column + bias
  trick) so the softmax denominator falls out of the same matmul; the
  reciprocal is broadcast across partitions with a K=1 matmul; out^T =
  Wo_slice.T @ ctx_norm^T with bo/4 added as a per-partition bias at PSUM
  eviction.
"""

import numpy as np
import ml_dtypes

import bass_rust
import concourse.bass as bass
import concourse.mybir as mybir
import concourse.tile as tile
from concourse.bass_utils import run_bass_kernel_spmd

B = 2
S = 2048
D = 1024
H = 16
DH = 64
OUT = 1024
NCORES = 8
HPC = H // 4  # heads per core = 4
E = HPC * DH  # 256 ctx dims per core
EV = HPC * (DH + 1)  # 260: v with interleaved ones columns

BF16 = mybir.dt.bfloat16
FP32 = mybir.dt.float32
FP16 = mybir.dt.float16

SCALE = 1.0 / float(np.sqrt(DH))


def _split_waits(nc, maxw=1):
    """This container's walrus rejects instructions carrying more than one
    semaphore wait ("Too many sync wait commands"); hoist extras onto
    standalone same-engine nops, preserving per-engine program order."""
    for bb in nc.main_func.blocks:
        new_il = []
        for inst in bb.instructions:
            si = inst.sync_info
            if si is not None and si.on_wait and len(si.on_wait) > maxw:
                waits = list(si.on_wait)
                for j, w in enumerate(waits[:-maxw]):
                    nop = mybir.InstNoOp(
                        name=f"{inst.name}-ws{j}", ins=[], outs=[], engine=inst.engine
                    )
                    nop.sync_info = bass_rust.SyncInfo(on_wait=[w], on_update=[])
                    new_il.append(nop)
                inst.sync_info = bass_rust.SyncInfo(
                    on_wait=waits[-maxw:], on_update=list(si.on_update)
                )
            new_il.append(inst)
        bb.instructions = new_il


def build_program():
    nc = bass.Bass()

    xT = nc.declare_dram_parameter("xT", [D, S], BF16, isOutput=False)
    wq = nc.declare_dram_parameter("wq", [D, E], BF16, isOutput=False)
    wk = nc.declare_dram_parameter("wk", [D, E], BF16, isOutput=False)
    wv = nc.declare_dram_parameter("wv", [D, EV], BF16, isOutput=False)
    wo = nc.declare_dram_parameter("wo", [E, OUT], BF16, isOutput=False)
    bqp = nc.declare_dram_parameter("bq", [128, E // 128], FP32, isOutput=False)
    bkp = nc.declare_dram_parameter("bk", [128, E // 128], FP32, isOutput=False)
    bvp = nc.declare_dram_parameter("bv", [1, EV], BF16, isOutput=False)
    bop = nc.declare_dram_parameter("bo4", [128, OUT // 128], FP32, isOutput=False)
    outT = nc.declare_dram_parameter("outT", [OUT, S], FP32, isOutput=True)

    KT = D // 128  # 8 k-tiles for projections
    MT = S // 128  # 16 token tiles
    NQ = S // 512  # 4 query slices of 512

    with tile.TileContext(nc) as tc:
        with (
            tc.tile_pool(name="w", bufs=1) as wpool,
            tc.tile_pool(name="work", bufs=3) as work,
            tc.tile_pool(name="cnp", bufs=1) as cnpool,
            tc.tile_pool(name="ps", bufs=2, space="PSUM") as psp,
            tc.tile_pool(name="ctxps", bufs=3, space="PSUM") as ctxp,
            tc.tile_pool(name="pop", bufs=1, space="PSUM") as pop,
        ):
            # ---- persistent SBUF residents ----
            xts = [wpool.tile([128, S], BF16, tag=f"xt{k}", name=f"xt{k}") for k in range(KT)]
            wqs = [wpool.tile([128, E], BF16, tag=f"wq{k}", name=f"wq{k}") for k in range(KT)]
            wks = [wpool.tile([128, E], BF16, tag=f"wk{k}", name=f"wk{k}") for k in range(KT)]
            wvs = [wpool.tile([128, EV], BF16, tag=f"wv{k}", name=f"wv{k}") for k in range(KT)]
            wos = [wpool.tile([128, OUT], BF16, tag=f"wo{k}", name=f"wo{k}") for k in range(2)]
            bq_s = wpool.tile([128, E // 128], FP32, tag="bq")
            bk_s = wpool.tile([128, E // 128], FP32, tag="bk")
            bv_s = wpool.tile([1, EV], BF16, tag="bv")
            bo_s = wpool.tile([128, OUT // 128], FP32, tag="bo")
            ones_b = wpool.tile([1, 128], BF16, tag="ones_b")
            ones_h = wpool.tile([1, 64], FP16, tag="ones_h")
            qts = [wpool.tile([128, S], BF16, tag=f"qt{m}", name=f"qt{m}") for m in range(2)]
            kts = [wpool.tile([128, S], BF16, tag=f"kt{m}", name=f"kt{m}") for m in range(2)]
            vts = [wpool.tile([128, EV], BF16, tag=f"vt{m}", name=f"vt{m}") for m in range(MT)]
            cns = [cnpool.tile([128, S], BF16, tag=f"cn{m}", name=f"cn{m}") for m in range(2)]

            # interleave so the first projection matmuls can start after the
            # first few transfers instead of after the whole 6 MiB load
            for k in range(KT):
                nc.sync.dma_start(out=wqs[k][:], in_=wq[k * 128 : (k + 1) * 128, :])
                nc.sync.dma_start(out=xts[k][:], in_=xT[k * 128 : (k + 1) * 128, :])
                nc.sync.dma_start(out=wks[k][:], in_=wk[k * 128 : (k + 1) * 128, :])
            nc.sync.dma_start(out=bq_s[:], in_=bqp[:])
            nc.sync.dma_start(out=bk_s[:], in_=bkp[:])
            for k in range(KT):
                nc.sync.dma_start(out=wvs[k][:], in_=wv[k * 128 : (k + 1) * 128, :])
            nc.sync.dma_start(out=bv_s[:], in_=bvp[:])
            for k in range(2):
                nc.sync.dma_start(out=wos[k][:], in_=wo[k * 128 : (k + 1) * 128, :])
            nc.sync.dma_start(out=bo_s[:], in_=bop[:])
            nc.vector.memset(ones_b[:], 1.0)
            nc.vector.memset(ones_h[:], 1.0)

            def qk_proj(hp):
                """qT/kT rows 128*hp..128*hp+127 (heads 2hp, 2hp+1), all tokens."""
                for w_s, dst, bias in ((wqs, qts, bq_s), (wks, kts, bk_s)):
                    for n in range(NQ):
                        ps = psp.tile([128, 512], FP32, tag="S", name="ps")
                        for k in range(KT):
                            nc.tensor.matmul(
                                ps[:],
                                lhsT=w_s[k][:, hp * 128 : (hp + 1) * 128],
                                rhs=xts[k][:, n * 512 : (n + 1) * 512],
                                start=(k == 0),
                                stop=(k == KT - 1),
                            )
                        nc.vector.tensor_scalar_add(
                            dst[hp][:, n * 512 : (n + 1) * 512],
                            ps[:],
                            bias[:, hp : hp + 1],
                        )

            def v_proj():
                for m in range(MT):
                    ps = pop.tile([128, 512], FP32, tag="po", name="psv")
                    for k in range(KT):
                        nc.tensor.matmul(
                            ps[:, :EV],
                            lhsT=xts[k][:, m * 128 : (m + 1) * 128],
                            rhs=wvs[k][:],
                            start=(k == 0),
                            stop=False,
                        )
                    # ones columns + v bias: rank-1 [1,128].T @ [1,EV]
                    nc.tensor.matmul(
                        ps[:, :EV], lhsT=ones_b[:], rhs=bv_s[:], start=False, stop=True
                    )
                    nc.scalar.copy(vts[m][:], ps[:, :EV])

            def normalize(ctx, hp, a, nq):
                """ctx [65,512] psum -> cn[hp] rows 64a..64a+63, cols nq*512.."""
                r = work.tile([1, 512], FP16, tag="recip")
                with nc.allow_low_precision(reason="fp16 recip feeds fp16 K=1 bcast matmul"):
                    nc.vector.reciprocal(r[:], ctx[64:65, :])
                bc = ctxp.tile([65, 512], FP32, tag="ctx")
                nc.tensor.matmul(
                    bc[0:64, :], lhsT=ones_h[:], rhs=r[:], start=True, stop=True
                )
                bcs = work.tile([64, 512], FP32, tag="bcs")
                nc.vector.tensor_copy(bcs[:], bc[0:64, :])
                nc.vector.tensor_mul(
                    cns[hp][64 * a : 64 * a + 64, nq * 512 : (nq + 1) * 512],
                    ctx[0:64, :],
                    bcs[:],
                )

            def out_proj_slice(n):
                """Partial out^T for query slice n (needs both cn tiles)."""
                for mo in range(OUT // 128):
                    ps = pop.tile([128, 512], FP32, tag="po", name="ps_o")
                    for k in range(2):
                        nc.tensor.matmul(
                            ps[:],
                            lhsT=wos[k][:, mo * 128 : (mo + 1) * 128],
                            rhs=cns[k][:, n * 512 : (n + 1) * 512],
                            start=(k == 0),
                            stop=(k == 1),
                        )
                    ot = work.tile([128, 512], FP32, tag="ot")
                    nc.vector.tensor_scalar_add(ot[:], ps[:], bo_s[:, mo : mo + 1])
                    nc.sync.dma_start(
                        out=outT[mo * 128 : (mo + 1) * 128, n * 512 : (n + 1) * 512],
                        in_=ot[:],
                    )

            def attention(hp):
                for nq in range(NQ):
                    ctx_a = ctxp.tile([65, 512], FP32, tag="ctx", name="ctx_a")
                    ctx_b = ctxp.tile([65, 512], FP32, tag="ctx", name="ctx_b")
                    for m in range(MT):
                        sps = psp.tile([128, 1024], FP32, tag="S", name="sps")
                        # scores^T for the head pair (row-packed K=64 matmuls)
                        nc.tensor.matmul(
                            sps[:, 0:512],
                            lhsT=kts[hp][0:64, m * 128 : (m + 1) * 128],
                            rhs=qts[hp][0:64, nq * 512 : (nq + 1) * 512],
                            start=True,
                            stop=True,
                        )
                        nc.tensor.matmul(
                            sps[:, 512:1024],
                            lhsT=kts[hp][64:128, m * 128 : (m + 1) * 128],
                            rhs=qts[hp][64:128, nq * 512 : (nq + 1) * 512],
                            start=True,
                            stop=True,
                        )
                        ee = work.tile([128, 1024], BF16, tag="E")
                        nc.scalar.activation(
                            ee[:], sps[:], mybir.ActivationFunctionType.Exp, scale=SCALE
                        )
                        ha = 2 * hp
                        nc.tensor.matmul(
                            ctx_a[:],
                            lhsT=vts[m][:, ha * 65 : ha * 65 + 65],
                            rhs=ee[:, 0:512],
                            start=(m == 0),
                            stop=(m == MT - 1),
                        )
                        nc.tensor.matmul(
                            ctx_b[:],
                            lhsT=vts[m][:, (ha + 1) * 65 : (ha + 1) * 65 + 65],
                            rhs=ee[:, 512:1024],
                            start=(m == 0),
                            stop=(m == MT - 1),
                        )
                    normalize(ctx_a, hp, 0, nq)
                    normalize(ctx_b, hp, 1, nq)
                    if hp == 1:
                        # both head-pairs' cn for slice nq are now complete;
                        # the out-projection overlaps attention of slice nq+1
                        out_proj_slice(nq)

            qk_proj(0)
            v_proj()
            attention(0)
            qk_proj(1)
            attention(1)

    _split_waits(nc)
    return nc


_PROGRAM = None


def _get_program():
    global _PROGRAM
    if _PROGRAM is None:
        _PROGRAM = build_program()
    return _PROGRAM


def _shard_inputs(x, Wq, bq, Wk, bk, Wv, bv, Wo, bo):
    bf16 = ml_dtypes.bfloat16
    in_maps = []
    for c in range(NCORES):
        b = c // 4
        g = c % 4
        hs = slice(g * HPC, (g + 1) * HPC)

        xT = np.ascontiguousarray(x[b].T).astype(bf16)  # [D, S]
        wq_c = np.ascontiguousarray(
            Wq[hs].transpose(1, 0, 2).reshape(D, E)
        ).astype(bf16)
        wk_c = np.ascontiguousarray(
            Wk[hs].transpose(1, 0, 2).reshape(D, E)
        ).astype(bf16)
        wv_c = np.zeros((D, EV), dtype=np.float32)
        bv_c = np.zeros((1, EV), dtype=np.float32)
        for h in range(HPC):
            wv_c[:, h * 65 : h * 65 + 64] = Wv[hs][h]
            bv_c[0, h * 65 : h * 65 + 64] = bv[hs][h]
            bv_c[0, h * 65 + 64] = 1.0
        wo_c = np.ascontiguousarray(Wo[g * E : (g + 1) * E, :]).astype(bf16)
        bq_c = np.ascontiguousarray(
            bq[hs].reshape(E // 128, 128).T
        ).astype(np.float32)
        bk_c = np.ascontiguousarray(
            bk[hs].reshape(E // 128, 128).T
        ).astype(np.float32)
        bo_c = np.ascontiguousarray(
            (bo.astype(np.float32) * 0.25).reshape(OUT // 128, 128).T
        ).astype(np.float32)

        in_maps.append(
            {
                "xT": xT,
                "wq": wq_c,
                "wk": wk_c,
                "wv": wv_c.astype(bf16),
                "wo": wo_c,
                "bq": bq_c,
                "bk": bk_c,
                "bv": bv_c.astype(bf16),
                "bo4": bo_c,
            }
        )
    return in_maps


def kernel(x, Wq, bq, Wk, bk, Wv, bv, Wo, bo, _trace=False, _result_box=None):
    x = np.asarray(x, dtype=np.float32)
    in_maps = _shard_inputs(
        np.asarray(x, np.float32),
        np.asarray(Wq, np.float32),
        np.asarray(bq, np.float32),
        np.asarray(Wk, np.float32),
        np.asarray(bk, np.float32),
        np.asarray(Wv, np.float32),
        np.asarray(bv, np.float32),
        np.asarray(Wo, np.float32),
        np.asarray(bo, np.float32),
    )
    nc = _get_program()
    res = run_bass_kernel_spmd(nc, in_maps, list(range(NCORES)), trace=_trace)
    if _result_box is not None:
        _result_box.append(res)

    out = np.empty((B, S, OUT), dtype=np.float32)
    for b in range(B):
        acc = res.results[4 * b]["outT"].astype(np.float32).copy()
        for g in range(1, 4):
            acc += res.results[4 * b + g]["outT"]
        out[b] = acc.T
    return out
